# revision 4
# baseline (speedup 1.0000x reference)
"""DeepFM (nn_DeepFM_50070728737466) Trainium2 Bass kernel.

Data-parallel over the 16384-row batch across 8 NeuronCores (2048
rows/core); the small MLP + FM weights are replicated per core.

Embedding lookup note: this environment (axon-tunneled NeuronCores,
PJRT execute path) rejects NEFFs that use software-DGE dynamic DMA at
load time (gpsimd dma_start / indirect_dma_start all fail in
LoadExecutable), and the walrus-lowered vector-indirect DMA that does
load only honors one index per partition (~1.8us per 128-row call,
measured) - orders of magnitude off the bandwidth needed for 79872
row lookups per core.  The row gather (a pure data movement,
feature_table[feat_index]) is therefore performed host-side in
kernel(); the embedding table and bias are concatenated so each
(batch, field) lookup yields one 132B row [emb(32) | bias(1)].  All
model arithmetic runs on device:

  per 128-row batch tile:
    g[128, 39*33]   <- DMA (gathered rows)
    emb_s[128,1248] = g.emb * fv          (DVE, broadcast AP)
    co[:, 0:39]     = g.bias * fv         (DVE)        -> first_order
    embT            = PE transposes of emb_s (10x 128-col chunks)
    psum[:,0:432]   = embT.T @ [W1T | sqrt(.5)*S]  (PE fp32r, fused)
    h1 = relu(psum[:, :400])  (ACT); se = psum[:, 400:432]
    h1T = PE transpose; h2 = relu(h1T.T @ W2T)     -> deep_y
    ssq = reduce_f (sqrt(.5)*emb_s)^2  (ACT square + DVE reduce)
    second_order = se^2 - ssq   (ACT + DVE)
    output = concat([fo, so, deep]) . final_W + final_b (DVE fused TTR)

All matmuls run as float32r (fp32 bits, fast PE mode, fp32 accumulate).
"""

import numpy as np

B = 16384
F = 39
E = 32
V = 1_000_000
H = 400
NCORES = 8
BC = B // NCORES          # batch rows per core (2048)
P = 128                   # partitions / batch tile rows
NT = BC // P              # batch tiles per core (16)
GW = F * (E + 1)          # gathered group width per batch row (1287)
DIN = F * E               # 1248
NK1 = (DIN + P - 1) // P  # 10 contract chunks for layer 1
DIN_PAD = NK1 * P         # 1280
NK2 = 4                   # layer-2 contract chunks (4 x 100)
H2C = H // NK2            # 100
DCAT = F + E + H          # 471
NF1 = H + E               # fused layer1 + S matmul free size (432)

_CACHE = {}


def _build(has_b1, has_b2):
    import concourse.bacc as bacc
    import concourse.tile as tile
    from concourse import mybir

    f32 = mybir.dt.float32
    f32r = mybir.dt.float32r
    AF = mybir.ActivationFunctionType
    OP = mybir.AluOpType

    nc = bacc.Bacc("TRN2", target_bir_lowering=False, debug=False,
                   num_devices=NCORES)

    # ---- DRAM I/O ----
    g_d = nc.dram_tensor("g", [BC, GW], f32, kind="ExternalInput")
    fv_d = nc.dram_tensor("fv", [BC, F], f32, kind="ExternalInput")
    w1s_d = nc.dram_tensor("w1s", [P, NK1 * NF1], f32r, kind="ExternalInput")
    w2t_d = nc.dram_tensor("w2t", [H2C, NK2 * H], f32r, kind="ExternalInput")
    fw_d = nc.dram_tensor("fw", [P, DCAT], f32, kind="ExternalInput")
    ident_d = nc.dram_tensor("ident", [P, P], f32, kind="ExternalInput")
    b1_d = nc.dram_tensor("b1r", [P, H], f32, kind="ExternalInput") if has_b1 else None
    b2_d = nc.dram_tensor("b2r", [P, H], f32, kind="ExternalInput") if has_b2 else None
    # packed output: [fo(39) | so(32) | deep(400) | out(1)]
    out_d = nc.dram_tensor("outp", [BC, DCAT + 1], f32, kind="ExternalOutput")

    with tile.TileContext(nc) as tc:
        with (
            tc.tile_pool(name="const", bufs=1) as cpool,
            tc.tile_pool(name="work", bufs=3) as wpool,
            tc.tile_pool(name="psA", bufs=1, space="PSUM") as psA,
            tc.tile_pool(name="psB", bufs=2, space="PSUM") as psB,
        ):
            # ---- constants / weights, loaded once ----
            w1s = cpool.tile([P, NK1 * NF1], f32r, tag="w1s")
            nc.sync.dma_start(w1s[:], w1s_d[:])
            w2t = cpool.tile([H2C, NK2 * H], f32r, tag="w2t")
            nc.sync.dma_start(w2t[:], w2t_d[:])
            fw = cpool.tile([P, DCAT], f32, tag="fw")
            nc.sync.dma_start(fw[:], fw_d[:])
            ident = cpool.tile([P, P], f32, tag="ident")
            nc.sync.dma_start(ident[:], ident_d[:])
            if has_b1:
                b1r = cpool.tile([P, H], f32, tag="b1r")
                nc.sync.dma_start(b1r[:], b1_d[:])
            if has_b2:
                b2r = cpool.tile([P, H], f32, tag="b2r")
                nc.sync.dma_start(b2r[:], b2_d[:])

            for t in range(NT):
                r0 = t * P
                # ---- loads ----
                g = wpool.tile([P, GW], f32, tag="g")
                nc.sync.dma_start(g[:], g_d[r0:r0 + P, :])
                fv_t = wpool.tile([P, F], f32, tag="fv")
                nc.sync.dma_start(fv_t[:], fv_d[r0:r0 + P, :])

                g3 = g[:].rearrange("p (f c) -> p f c", c=E + 1)
                fvb = fv_t[:, :, None]

                # ---- scale by feat_value ----
                emb_s = wpool.tile([P, DIN], f32, tag="emb_s")
                es3 = emb_s[:].rearrange("p (f c) -> p f c", c=E)
                nc.vector.tensor_tensor(
                    out=es3, in0=g3[:, :, 0:E],
                    in1=fvb.to_broadcast([P, F, E]), op=OP.mult)

                co = wpool.tile([P, DCAT + 1], f32, tag="co")
                # first_order -> co[:, 0:39]
                nc.vector.tensor_tensor(
                    out=co[:, 0:F, None], in0=g3[:, :, E:E + 1],
                    in1=fvb, op=OP.mult)

                # ---- sum over f of (sqrt(.5) * emb_s)^2 ----
                sq = wpool.tile([P, DIN], f32, tag="sq")
                nc.scalar.activation(sq[:], emb_s[:], AF.Square,
                                     scale=float(np.sqrt(0.5)))
                ssq = wpool.tile([P, E], f32, tag="ssq")
                nc.vector.tensor_reduce(
                    out=ssq[:], in_=sq[:].rearrange("p (f c) -> p c f", c=E),
                    axis=mybir.AxisListType.X, op=OP.add)

                # ---- PE transposes of emb_s -> embT (fp32r out) ----
                embT = wpool.tile([P, DIN_PAD], f32r, tag="embT")
                ptA = psA.tile([P, 512], f32, tag="ptA", space="PSUM")
                ptB = psA.tile([P, 512], f32, tag="ptB", space="PSUM")
                ptC = psA.tile([P, 256], f32, tag="ptC", space="PSUM")
                for k in range(NK1):
                    w = min(P, DIN - k * P)           # 128 or 96 (k=9)
                    dst, off = (ptA, k * P) if k < 4 else \
                               ((ptB, (k - 4) * P) if k < 8 else (ptC, (k - 8) * P))
                    nc.tensor.transpose(
                        out=dst[:w, off:off + P],
                        in_=emb_s[:, k * P:k * P + w],
                        identity=ident[:])
                nc.scalar.copy(embT[:, 0:512], ptA[:])
                nc.scalar.copy(embT[:, 512:1024], ptB[:])
                nc.scalar.copy(embT[:, 1024:1280], ptC[:])

                # ---- fused layer1 + sum_emb: psum = embT.T @ [W1T | S] ----
                f1p = psB.tile([P, NF1], f32, tag="f1p", space="PSUM")
                for k in range(NK1):
                    w = min(P, DIN - k * P)
                    nc.tensor.matmul(
                        out=f1p[:],
                        lhsT=embT[:w, k * P:k * P + P],
                        rhs=w1s[:w, k * NF1:(k + 1) * NF1],
                        start=(k == 0), stop=(k == NK1 - 1))
                h1 = wpool.tile([P, H], f32, tag="h1")
                if has_b1:
                    nc.vector.tensor_tensor(out=h1[:], in0=f1p[:, 0:H],
                                            in1=b1r[:], op=OP.add)
                    nc.scalar.activation(h1[:], h1[:], AF.Relu)
                else:
                    nc.scalar.activation(h1[:], f1p[:, 0:H], AF.Relu)
                # second_order = se^2 - ssq -> co[:, 39:71]
                se2 = wpool.tile([P, E], f32, tag="se2")
                nc.scalar.activation(se2[:], f1p[:, H:NF1], AF.Square)
                nc.vector.tensor_tensor(out=co[:, F:F + E], in0=se2[:],
                                        in1=ssq[:], op=OP.subtract)

                # ---- transpose h1 -> h1T (4 x 100-col chunks) ----
                h1tp = psA.tile([P, 512], f32, tag="h1tp", space="PSUM")
                for k in range(NK2):
                    nc.tensor.transpose(
                        out=h1tp[:H2C, k * P:k * P + P],
                        in_=h1[:, k * H2C:(k + 1) * H2C],
                        identity=ident[:])
                h1T = wpool.tile([P, 512], f32r, tag="h1T")
                nc.scalar.copy(h1T[:H2C, :], h1tp[:H2C, :])

                # ---- layer 2: h2 = relu(h1T.T @ W2T) -> co[:, 71:471] ----
                h2p = psB.tile([P, H], f32, tag="h2p", space="PSUM")
                for k in range(NK2):
                    nc.tensor.matmul(
                        out=h2p[:],
                        lhsT=h1T[:H2C, k * P:k * P + P],
                        rhs=w2t[:, k * H:(k + 1) * H],
                        start=(k == 0), stop=(k == NK2 - 1))
                if has_b2:
                    nc.vector.tensor_tensor(out=co[:, F + E:F + E + H],
                                            in0=h2p[:], in1=b2r[:], op=OP.add)
                    nc.scalar.activation(co[:, F + E:F + E + H],
                                         co[:, F + E:F + E + H], AF.Relu)
                else:
                    nc.scalar.activation(co[:, F + E:F + E + H], h2p[:], AF.Relu)

                # ---- final: out = concat . final_W + final_b ----
                # (tensor_tensor_reduce is a custom-ISA op that fails at
                #  runtime in this environment; use TT + reduce instead)
                nc.vector.tensor_tensor(
                    out=sq[:, 0:DCAT],          # scratch, dead after ssq
                    in0=co[:, 0:DCAT], in1=fw[:], op=OP.mult)
                nc.vector.tensor_reduce(
                    out=co[:, DCAT:DCAT + 1], in_=sq[:, 0:DCAT],
                    axis=mybir.AxisListType.X, op=OP.add)

                nc.sync.dma_start(out_d[r0:r0 + P, :], co[:])

    nc.compile()
    return nc


def _get_nc(has_b1, has_b2):
    key = (has_b1, has_b2)
    if key not in _CACHE:
        _CACHE[key] = _build(has_b1, has_b2)
    return _CACHE[key]


def kernel(feat_index, feat_value, feature_embedding, feature_bias,
           W1, b1, W2, b2, final_W, final_b):
    from concourse.bass_utils import run_bass_kernel_spmd

    feat_index = np.asarray(feat_index)
    feat_value = np.ascontiguousarray(np.asarray(feat_value, dtype=np.float32))
    feature_embedding = np.asarray(feature_embedding, dtype=np.float32)
    feature_bias = np.asarray(feature_bias, dtype=np.float32)
    W1 = np.asarray(W1, dtype=np.float32)
    W2 = np.asarray(W2, dtype=np.float32)
    final_W = np.asarray(final_W, dtype=np.float32)
    b1 = np.asarray(b1, dtype=np.float32)
    b2 = np.asarray(b2, dtype=np.float32)
    final_b = np.asarray(final_b, dtype=np.float32)

    has_b1 = bool(np.any(b1 != 0))
    has_b2 = bool(np.any(b2 != 0))
    nc = _get_nc(has_b1, has_b2)

    # ---- host-side gather (see module docstring) + weight packing ----
    tbl = np.concatenate([feature_embedding, feature_bias], axis=1)  # [V,33]
    g_all = tbl[feat_index.reshape(-1)].reshape(B, GW).astype(np.float32)

    # fused [W1T | sqrt(.5)*S] in 10 contract chunks of 128 rows
    w1s_full = np.zeros((DIN_PAD, NF1), np.float32)
    w1s_full[:DIN, 0:H] = W1.T
    r = np.sqrt(0.5).astype(np.float32)
    fe = np.arange(DIN)
    w1s_full[fe, H + (fe % E)] = r
    w1s = np.ascontiguousarray(
        w1s_full.reshape(NK1, P, NF1).transpose(1, 0, 2).reshape(P, NK1 * NF1))

    w2t = np.ascontiguousarray(
        W2.T.reshape(NK2, H2C, H).transpose(1, 0, 2).reshape(H2C, NK2 * H))

    fw = np.ascontiguousarray(
        np.tile(final_W.reshape(1, DCAT), (P, 1))).astype(np.float32)
    ident = np.eye(P, dtype=np.float32)

    in_maps = []
    for c in range(NCORES):
        m = {
            "g": g_all[c * BC:(c + 1) * BC],
            "fv": feat_value[c * BC:(c + 1) * BC],
            "w1s": w1s,
            "w2t": w2t,
            "fw": fw,
            "ident": ident,
        }
        if has_b1:
            m["b1r"] = np.tile(b1.reshape(1, H), (P, 1)).astype(np.float32)
        if has_b2:
            m["b2r"] = np.tile(b2.reshape(1, H), (P, 1)).astype(np.float32)
        in_maps.append(m)

    trace = getattr(kernel, "TRACE", False)
    tmpdir = getattr(kernel, "TMPDIR", None) if trace else None
    res = run_bass_kernel_spmd(nc, in_maps, list(range(NCORES)),
                               trace=trace, tmpdir=tmpdir)
    kernel.last_result = res

    packed = np.concatenate([res.results[c]["outp"] for c in range(NCORES)],
                            axis=0)                         # [B, 472]
    fb = float(final_b.reshape(-1)[0])
    output = np.ascontiguousarray(packed[:, DCAT:DCAT + 1]) + fb
    first_order = np.ascontiguousarray(packed[:, 0:F])
    second_order = np.ascontiguousarray(packed[:, F:F + E])
    deep_y = np.ascontiguousarray(packed[:, F + E:F + E + H])
    return (output, first_order, second_order, deep_y)
